# revision 18
# baseline (speedup 1.0000x reference)
"""Multi-head attention (B=2, T=2048, d_model=1024, H=16, hd=64) on 8 Trainium2
NeuronCores.

Sharding: the 32 (batch, head) attention units are split as 4 consecutive heads
of one batch per core (core c -> batch c//4, heads 4*(c%4) .. 4*(c%4)+3). Each
core computes its own QKV projection slice, causal attention for its heads, and
a partial out-projection (its 256 rows of W_out). The host sums the 4 partials
per batch and adds the output bias.

Device-side layout:
  qT/kT [hd, T]  <- lhsT=W slice, rhs=xT   (per-head halves in partitions)
  v     [T, hd]  (+ ones column for the row-sum trick)
  sT    [k, q]   <- K=64 row-tiled matmul PAIRS: head 2m in PE rows 0-63,
                    head 2m+1 in rows 64-127, issued back-to-back so the two
                    heads' score matmuls run CONCURRENTLY in disjoint row
                    strips (~2x score throughput vs zero-padded full-K).
  E     [k, q]   <- exp(sT/sqrt(hd)) on ScalarE; causal masking of diagonal
                    blocks AFTER exp by zeroing on the idle GPSIMD engine.
  pv    [q, 4, hd+1] <- lhsT=E chunk, rhs=[v|1]; 4 q-tile accumulation chains
                    share one PSUM bank; normalization is one reciprocal +
                    one broadcast multiply per (head, q-group).
  aT    [hd, T]  via DRAM round-trip DMA transpose (off the critical engines)
  out  += aT.T @ W_out slice  (partial, fp32)

Exact bias folds (no accuracy cost):
  - k bias: (q+bq)@bk is constant along the softmax axis -> dropped entirely.
  - v bias: attention weights sum to 1, so out = a_nb@W_out + bv@W_out; the
    constant row bv@W_out is added on the host together with b_out.

Scheduling: the PE executes its queue in emission order, and ScalarE's ~85us
of exp is the near-critical path, so the emission interleaves one "filler"
unit of independent PE work (a projection tile, a v tile, or an out-projection
chunk) after every scores tile-group. Fillers allocate from their own PSUM
pool so they never queue behind the exp that frees a scores slot (that FIFO
coupling caused 45% HAM-throttle in an earlier version). Head-pair 0 walks
q-groups ascending (first exp at ~10us), head-pair 1 descending with each
group's out-projection queued as fillers one group later, so the kernel tail
is only the smallest group's out-projection.
"""

import math
import os
from contextlib import ExitStack
from dataclasses import dataclass

import numpy as np
import ml_dtypes

import concourse.bass as bass
import concourse.tile as tile
from concourse import bacc, mybir
from concourse import bass_utils

AF = mybir.ActivationFunctionType
ALU = mybir.AluOpType
DT = mybir.dt

N_CORES = 8


@dataclass(frozen=True)
class Cfg:
    T: int = 2048        # sequence length
    DM: int = 1024       # d_model
    HD: int = 64         # head dim
    NH: int = 4          # heads per core
    mode: str = "causal"  # "causal" | "full"
    mm: str = "bf16"     # matmul operand dtype: "bf16" | "f32r" | "f32"

    @property
    def NHD(self):
        return self.NH * self.HD          # qkv slice width per core

    @property
    def KC(self):
        return self.DM // 128             # contraction chunks for projections

    @property
    def MC(self):
        return self.NHD // 128            # qT/kT partition chunks

    @property
    def TC(self):
        return self.T // 128              # t chunks

    @property
    def QW(self):
        return min(512, self.T)           # q group width

    @property
    def QG(self):
        return self.T // self.QW

    @property
    def QT(self):
        return self.QW // 128             # q tiles per group

    @property
    def mmdt(self):
        return {"bf16": DT.bfloat16, "f32r": DT.float32r, "f32": DT.float32}[self.mm]

    @property
    def npmm(self):
        return ml_dtypes.bfloat16 if self.mm == "bf16" else np.float32


def build_program(cfg: Cfg):
    """Build + compile the SPMD single-core program. Returns (nc, input_names)."""
    c = cfg
    assert c.DM % 128 == 0 and c.NHD % 128 == 0 and c.T % 512 == 0
    assert c.mode in ("causal", "full")
    nc = bacc.Bacc("TRN2", target_bir_lowering=False, debug=False,
                   num_devices=N_CORES)
    f32 = DT.float32
    mmdt = c.mmdt

    xT = nc.dram_tensor("xT", [c.DM, c.T], mmdt, kind="ExternalInput").ap()
    wq = nc.dram_tensor("wq", [c.DM, c.NHD], mmdt, kind="ExternalInput").ap()
    wk = nc.dram_tensor("wk", [c.DM, c.NHD], mmdt, kind="ExternalInput").ap()
    wv = nc.dram_tensor("wv", [c.DM, c.NHD], mmdt, kind="ExternalInput").ap()
    bq = nc.dram_tensor("bq", [128, c.MC], f32, kind="ExternalInput").ap()
    wo = nc.dram_tensor("wo", [c.NHD, c.DM], mmdt, kind="ExternalInput").ap()
    out = nc.dram_tensor("out", [c.T, c.DM], DT.bfloat16,
                     kind="ExternalOutput").ap()

    with tile.TileContext(nc) as tc, ExitStack() as ctx:
        _body(ctx, tc, c, xT, wq, wk, wv, bq, wo, out)
    nc.compile()
    names = ["xT", "wq", "wk", "wv", "bq", "wo"]
    return nc, names


def _body(ctx, tc, c: Cfg, xT, wq, wk, wv, bq, wo, out):
    nc = tc.nc
    f32 = DT.float32
    bf16 = DT.bfloat16
    mmdt = c.mmdt
    causal = c.mode == "causal"
    scale = 1.0 / math.sqrt(c.HD)
    HD1 = c.HD + 1

    const = ctx.enter_context(tc.tile_pool(name="const", bufs=1))
    big = ctx.enter_context(tc.tile_pool(name="big", bufs=1))
    epool = ctx.enter_context(tc.tile_pool(name="E", bufs=18))
    rpool = ctx.enter_context(tc.tile_pool(name="r", bufs=4))
    # PSUM budget (8 banks): one shared pool 3x[128,1024] (6 banks) for
    # scores / projections / out-projection, + pv 2x[128,4*65] (2 banks).
    # The emission pattern is [scores,scores,filler] so the slot FIFO gives
    # ~1.5 tile-groups of lookahead and filler copies (fast, DVE) never gate
    # the scores stream for long.
    ps_s = ctx.enter_context(tc.tile_pool(name="pss", bufs=3, space="PSUM"))
    ps_f = ps_s
    ps_pv = ctx.enter_context(tc.tile_pool(name="pspv", bufs=2, space="PSUM"))
    dramp = ctx.enter_context(tc.tile_pool(name="dram", bufs=1, space="DRAM"))
    ostage = ctx.enter_context(tc.tile_pool(name="ostage", bufs=4))

    # ---- input DMAs, ordered so the first projection matmuls start early ----
    wk_sb = big.tile([128, c.KC, c.NHD], mmdt, tag="wk")
    nc.sync.dma_start(out=wk_sb[:], in_=wk.rearrange("(c p) n -> p c n", p=128))
    wq_sb = big.tile([128, c.KC, c.NHD], mmdt, tag="wq")
    nc.sync.dma_start(out=wq_sb[:], in_=wq.rearrange("(c p) n -> p c n", p=128))

    xT_sb = big.tile([128, c.KC, c.T], mmdt, tag="xT")
    xTd = xT.rearrange("(c p) t -> p c t", p=128)
    for lo, hi in ((0, 512), (512, c.T)):
        nc.sync.dma_start(out=xT_sb[:, :, lo:hi], in_=xTd[:, :, lo:hi])

    bq_sb = const.tile([128, c.MC], f32, tag="bq")
    nc.sync.dma_start(out=bq_sb[:], in_=bq)

    wv_sb = big.tile([128, c.KC, c.NHD], mmdt, tag="wv")
    nc.sync.dma_start(out=wv_sb[:], in_=wv.rearrange("(c p) n -> p c n", p=128))
    wo_sb = big.tile([128, c.MC, c.DM], mmdt, tag="wo")
    nc.sync.dma_start(out=wo_sb[:],
                      in_=wo.rearrange("(c p) n -> p c n", p=128))

    # ---- persistent SBUF tensors ----
    qT_z = big.tile([128, c.NH, c.T], mmdt, tag="qT")
    kT_sb = big.tile([128, c.MC, c.T], mmdt, tag="kT")
    v_sb = big.tile([128, c.TC, c.NH, HD1], bf16, tag="v")
    nc.vector.memset(v_sb[:, :, :, c.HD:HD1], 1.0)
    a_sb = big.tile([128, c.TC, c.NH, c.HD], bf16, tag="a")
    a_dram = dramp.tile([c.T, c.NHD], bf16, tag="adram")
    aT_sb = big.tile([128, c.MC, c.T], bf16, tag="aT")

    W2 = 512                               # projection tile width

    # ---- filler units (~1-2us of independent PE work each) ----
    def emit_k_tile(m, n):
        ps = ps_f.tile([128, W2], f32, tag="mm", name="ps_k")
        for k in range(c.KC):
            nc.tensor.matmul(
                ps[:],
                lhsT=wk_sb[:, k, m * 128:(m + 1) * 128],
                rhs=xT_sb[:, k, n * W2:(n + 1) * W2],
                start=(k == 0), stop=(k == c.KC - 1),
            )
        nc.vector.tensor_copy(kT_sb[:, m, n * W2:(n + 1) * W2], ps[:])

    def emit_q_tile(m, n):
        ps = ps_f.tile([128, W2], f32, tag="mm", name="ps_q")
        for k in range(c.KC):
            nc.tensor.matmul(
                ps[:],
                lhsT=wq_sb[:, k, m * 128:(m + 1) * 128],
                rhs=xT_sb[:, k, n * W2:(n + 1) * W2],
                start=(k == 0), stop=(k == c.KC - 1),
            )
        sl = slice(n * W2, (n + 1) * W2)
        nc.vector.tensor_scalar_add(
            qT_z[0:64, 2 * m, sl], ps[0:64, :], bq_sb[0:64, m:m + 1])
        nc.vector.tensor_scalar_add(
            qT_z[64:128, 2 * m + 1, sl], ps[64:128, :], bq_sb[64:128, m:m + 1])

    def emit_v_tile(t):
        # one t-chunk of v, normal layout (bias folded into host output bias)
        ps = ps_f.tile([128, c.NHD], f32, tag="mm", name="ps_v")
        for k in range(c.KC):
            nc.tensor.matmul(
                ps[:],
                lhsT=xT_sb[:, k, t * 128:(t + 1) * 128],
                rhs=wv_sb[:, k, :],
                start=(k == 0), stop=(k == c.KC - 1),
            )
        nc.vector.tensor_copy(
            v_sb[:, t, :, 0:c.HD],
            ps[:].rearrange("p (h e) -> p h e", e=c.HD),
        )

    def emit_o_tile(t, tail=False):
        # out-projection for one t-chunk
        ps = ps_f.tile([128, c.DM], f32, tag="mm", name="ps_o")
        for d in range(c.DM // 512):
            for ci in range(c.MC):
                nc.tensor.matmul(
                    ps[:, d * 512:(d + 1) * 512],
                    lhsT=aT_sb[:, ci, t * 128:(t + 1) * 128],
                    rhs=wo_sb[:, ci, d * 512:(d + 1) * 512],
                    start=(ci == 0), stop=(ci == c.MC - 1),
                )
        ot = ostage.tile([128, c.DM], bf16, tag="o")
        # in the tail ScalarE is done with exps: alternate copies across
        # ScalarE and VectorE so psum slots recycle twice as fast
        if tail and t % 2 == 0:
            nc.scalar.copy(ot[:], ps[:])
        else:
            nc.vector.tensor_copy(ot[:], ps[:])
        nc.sync.dma_start(out=out[t * 128:(t + 1) * 128, :], in_=ot[:])

    # tagged filler queue: pace() pops one unit; drain(pred) pops from the
    # front until no queued unit matches pred (used for dependency barriers)
    queue: list = []                      # items: (tag_tuple, thunk)

    def drip():
        # tiny matmul to keep the PE activity monitor from re-throttling
        # the clock to 1.2 GHz during exp-bound stretches
        ps = ps_f.tile([128, 16], f32, tag="mm", name="ps_drip")
        nc.tensor.matmul(ps[0:16, 0:16], lhsT=wk_sb[:, 0, 0:16],
                         rhs=wk_sb[:, 0, 16:32], start=True, stop=True)

    def pace():
        if queue:
            queue.pop(0)[1]()
        else:
            drip()

    def drain(pred):
        rest = []
        for tag, thunk in queue:
            if pred(tag):
                thunk()
            else:
                rest.append((tag, thunk))
        queue[:] = rest

    # ---- attention for one head pair over one q-group ----
    def attn_group(hp, g):
        kmax = (g + 1) * c.QT if causal else c.TC
        # projections this group's scores read must be complete
        drain(lambda tag: tag[0] in ("k", "q") and tag[1] == hp
              and ((tag[2] <= g) if causal else True))
        # One [128,1024] psum tile per k-chunk holds BOTH heads' scores
        # (h0 in bank 0, h1 in bank 1): the pair of K=64 row-tiled matmuls
        # shares one slot-release semaphore, so they issue back-to-back and
        # run concurrently in disjoint PE row strips; one exp covers both.
        etiles = []                         # per k-chunk: E tile [128,2*512]
        for kc in range(kmax):
            jj = kc - g * c.QT if causal else -1
            off = jj * 128 if jj >= 0 else 0
            ps = ps_s.tile([128, 2, 512], f32, tag="mm", name="ps_s")
            for hl in range(2):
                h = 2 * hp + hl
                b0 = hl * 64
                nc.tensor.matmul(
                    ps[:, hl, off:512],
                    lhsT=kT_sb[b0:b0 + 64, hp, kc * 128:(kc + 1) * 128],
                    rhs=qT_z[b0:b0 + 64, h, g * c.QW + off:(g + 1) * c.QW],
                    start=True, stop=True,
                )
            et = epool.tile([128, 2, 512], bf16, tag="E", name="et")
            nc.scalar.activation(
                et[:, :, off:512], ps[:, :, off:512], AF.Exp, scale=scale,
            )
            # zero the masked upper triangle of diagonal blocks on the
            # otherwise idle GPSIMD engine
            if jj >= 0:
                for hl in range(2):
                    blk = et[:, hl, jj * 128:(jj + 1) * 128]
                    nc.gpsimd.affine_select(
                        out=blk, in_=blk,
                        compare_op=ALU.is_ge, fill=0.0,
                        base=0, channel_multiplier=-1, pattern=[[1, 128]],
                    )
            etiles.append(et)
            if kc % 2 == 1:
                pace()                     # keep PE busy while exps drain
            else:
                nc.tensor.ldweights(weights=kT_sb[:, 0, 0:128])  # HAM drip

        # PV needs v chunks < kmax
        drain(lambda tag: tag[0] == "v" and tag[1] < kmax)

        # PV: 4 accumulation chains (one per q-tile) share one PSUM bank
        for hl in range(2):
            h = 2 * hp + hl
            pv = ps_pv.tile([128, c.QT, HD1], f32, tag="pv", name="ps_pv")
            for j in range(c.QT):
                qt = g * c.QT + j
                kn = qt + 1 if causal else c.TC
                for kc in range(kn):
                    nc.tensor.matmul(
                        pv[:, j, :],
                        lhsT=etiles[kc][:, hl, j * 128:(j + 1) * 128],
                        rhs=v_sb[:, kc, h, :],
                        start=(kc == 0), stop=(kc == kn - 1),
                    )
            r = rpool.tile([128, c.QT, 1], f32, tag="r")
            nc.vector.reciprocal(r[:, :, 0], pv[:, :, c.HD])
            nc.vector.tensor_tensor(
                out=a_sb[:, g * c.QT:(g + 1) * c.QT, h, :],
                in0=pv[:, :, 0:c.HD],
                in1=r.broadcast_to([128, c.QT, c.HD]),
                op=ALU.mult,
            )
            pace()

    # transpose a -> aT for one q-group via DRAM round trip (DMA engines)
    def emit_transpose(g):
        nc.sync.dma_start(
            out=a_dram[g * c.QW:(g + 1) * c.QW, :].rearrange(
                "(q p) n -> p q n", p=128),
            in_=a_sb[:, g * c.QT:(g + 1) * c.QT, :, :],
        )
        for ci in range(c.MC):
            nc.sync.dma_start(
                out=aT_sb[:, ci, g * c.QW:(g + 1) * c.QW],
                in_=a_dram[g * c.QW:(g + 1) * c.QW, ci * 128:(ci + 1) * 128],
                transpose=True,
            )

    # ---- emission schedule ----
    # PE warm-up: a dense burst of tiny matmuls (operand values irrelevant)
    # during the input-DMA wait, so the HAM clock-gate reaches 8/8 before
    # the first real matmuls instead of running them at 1.2 GHz
    wps = ps_f.tile([128, 16], f32, tag="mm", name="ps_warm")
    for _ in range(160):
        nc.tensor.matmul(wps[0:16, 0:16], lhsT=wk_sb[:, 0, 0:16],
                         rhs=wk_sb[:, 0, 16:32], start=True, stop=True)

    # eager: just enough projection for (hp0, g0); everything else is queued
    # as fillers paced into the attention stream.
    emit_k_tile(0, 0)
    emit_q_tile(0, 0)

    nT = c.T // W2
    # per-segment interleave: each "round" queues the projections the NEXT
    # segments' scores will need (one notch ahead of the group sequence),
    # then the v tiles the upcoming PV stage reads; pace() pops in order,
    # selective drains are the correctness backstop
    for n in range(1, nT):
        queue.append((("k", 0, n), lambda n=n: emit_k_tile(0, n)))
        queue.append((("q", 0, n), lambda n=n: emit_q_tile(0, n)))
        queue.append((("k", 1, n - 1), lambda n=n: emit_k_tile(1, n - 1)))
        queue.append((("q", 1, n - 1), lambda n=n: emit_q_tile(1, n - 1)))
        for t in range(4 * (n - 1), 4 * n):
            queue.append((("v", t), lambda t=t: emit_v_tile(t)))
    queue.append((("k", 1, nT - 1), lambda: emit_k_tile(1, nT - 1)))
    queue.append((("q", 1, nT - 1), lambda: emit_q_tile(1, nT - 1)))
    for t in range(4 * (nT - 1), 4 * nT):
        queue.append((("v", t), lambda t=t: emit_v_tile(t)))

    # head pairs interleaved so the exp backlog : PE work ratio stays even
    # across the whole kernel (an all-hp0-then-all-hp1 order leaves the back
    # half exp-bound, which starves the PE into HAM-throttled half-clock).
    # The last segment is the smallest group so the tail is short. Each hp1
    # group's out-projection is queued as fillers TWO segments later so its
    # aT transpose DMAs are done before its matmuls enter the shared psum
    # FIFO (a waiting out-proj matmul stalls everything behind it).
    seq = [(0, 0), (0, 1), (1, 1), (0, 2), (1, 2), (0, 3), (1, 3), (1, 0)] \
        if causal else \
        [(0, 0), (1, 0), (0, 1), (1, 1), (0, 2), (1, 2), (0, 3), (1, 3)]
    o_new, o_ready = [], []
    for hp, g in seq:
        for og in o_ready:
            for t in range(og * c.QT, (og + 1) * c.QT):
                queue.append((("o", t), lambda t=t: emit_o_tile(t)))
        o_ready, o_new = o_new, []
        attn_group(hp, g)
        if hp == 1:
            emit_transpose(g)
            o_new.append(g)
    drain(lambda tag: True)
    for og in o_ready + o_new:
        for t in range(og * c.QT, (og + 1) * c.QT):
            drip()
            emit_o_tile(t, tail=True)


# ---------------------------------------------------------------------------
# host side
# ---------------------------------------------------------------------------

_CACHE: dict = {}


def _get_program(cfg: Cfg):
    key = cfg
    if key not in _CACHE:
        _CACHE[key] = build_program(cfg)
    return _CACHE[key]


def _mask_mode(mask: np.ndarray, T: int) -> str:
    m = (np.asarray(mask).reshape(T, T) != 0)
    if m.all():
        return "full"
    if np.array_equal(m, np.tril(np.ones((T, T), dtype=bool))):
        return "causal"
    raise NotImplementedError("only causal/full masks supported")


def make_in_maps(cfg: Cfg, x, W_qkv, b_qkv, W_out, mask=None):
    """Slice full inputs into the 8 per-core input dicts."""
    c = cfg
    npmm = c.npmm
    B = x.shape[0]
    n_hg = N_CORES // B                      # head groups per batch
    in_maps = []
    for core in range(N_CORES):
        b, hg = divmod(core, n_hg)
        col0 = hg * c.NHD
        xT_ = np.ascontiguousarray(x[b].T).astype(npmm)
        wq_ = np.ascontiguousarray(W_qkv[:, col0:col0 + c.NHD]).astype(npmm)
        wk_ = np.ascontiguousarray(
            W_qkv[:, c.DM + col0:c.DM + col0 + c.NHD]).astype(npmm)
        wv_ = np.ascontiguousarray(
            W_qkv[:, 2 * c.DM + col0:2 * c.DM + col0 + c.NHD]).astype(npmm)
        bq_ = np.ascontiguousarray(
            b_qkv[col0:col0 + c.NHD].reshape(c.MC, 128).T).astype(np.float32)
        wo_ = np.ascontiguousarray(W_out[col0:col0 + c.NHD, :]).astype(npmm)
        in_maps.append(dict(xT=xT_, wq=wq_, wk=wk_, wv=wv_, bq=bq_, wo=wo_))
    return in_maps


def run_sharded(cfg: Cfg, x, W_qkv, b_qkv, W_out, b_out, mask=None, **kw):
    """Run the SPMD program on 8 cores and assemble the full output."""
    nc, _names = _get_program(cfg)
    in_maps = make_in_maps(cfg, x, W_qkv, b_qkv, W_out, mask)
    res = bass_utils.run_bass_kernel_spmd(
        nc, in_maps, core_ids=list(range(N_CORES)), **kw,
    )
    outs = [np.asarray(r["out"], dtype=np.float32) for r in res.results]
    B = x.shape[0]
    n_hg = N_CORES // B
    # v bias folded here: attention weights sum to 1, so the v-bias term is
    # the constant row bv @ W_out
    bv = b_qkv[2 * cfg.DM:3 * cfg.DM].astype(np.float32)
    b_eff = b_out.astype(np.float32) + bv @ W_out.astype(np.float32)
    y = np.stack([
        np.sum(outs[b * n_hg:(b + 1) * n_hg], axis=0) for b in range(B)
    ]) + b_eff
    return y.astype(np.float32), res


def kernel(x, W_qkv, b_qkv, W_out, b_out, mask):
    x = np.asarray(x, dtype=np.float32)
    W_qkv = np.asarray(W_qkv, dtype=np.float32)
    b_qkv = np.asarray(b_qkv, dtype=np.float32)
    W_out = np.asarray(W_out, dtype=np.float32)
    b_out = np.asarray(b_out, dtype=np.float32)
    B, T, DM = x.shape
    mode = _mask_mode(mask, T)
    cfg = Cfg(T=T, DM=DM, mode=mode, mm=os.environ.get("MHA_MM_DT", "bf16"))
    y, _ = run_sharded(cfg, x, W_qkv, b_qkv, W_out, b_out, mask)
    return y
